# revision 3
# baseline (speedup 1.0000x reference)
# CenterLoss Trainium2 kernel.
#
# reference computes the full [B, C] squared-distance matrix but only reads
# the true-label entry of each row:
#   dist[i] = ||x[i]||^2 + ||centers[l_i]||^2 - 2 x[i].centers[l_i]
#           = ||x[i] - centers[l_i]||^2
#   loss = mean(clip(dist, 1e-12, 1e12))
# so the kernel only needs a per-sample gather of center rows plus a fused
# (x-c)^2 row reduction - no matmul, memory-bound.
#
# Sharding (host side, inside kernel()):
#   - sort samples by label; each of the 8 cores gets 256 consecutive
#     samples of the sorted order, so its labels live in a contiguous
#     class range [start_k, start_k + W).
#   - core k inputs: its 256 x-rows (gather-layout), the centers window
#     rows [start_k : start_k+W) (same W on all cores - SPMD), and the
#     window-relative labels as int32.
#   - device: indirect-DMA gather of the 256 center rows, d = x - c on
#     VectorE, Square+accumulate on ScalarE -> per-sample dist, clip,
#     write [128, GROUPS] dists per core.
#   - host: sum the 8 partial vectors, divide by B.

import os

import numpy as np

B = 2048
C = 16384
F = 2048
N_CORES = 8
SHARD = B // N_CORES  # 256 samples per core
P = 128
GROUPS = SHARD // P  # 2 gather groups of 128 rows

_prog_cache: dict = {}

# test.py introspection: the last BassKernelResults (exec_time_ns etc.)
LAST_RESULTS = None


def _build_program(w_rows: int):
    """One SPMD program, shared by all 8 cores; only the data differs."""
    import concourse.bacc as bacc
    import concourse.bass as bass
    import concourse.tile as tile
    from concourse import mybir

    nc = bacc.Bacc("TRN2", debug=False)
    xg = nc.dram_tensor("xg", [GROUPS, P, F], mybir.dt.float32, kind="ExternalInput")
    cw = nc.dram_tensor("cw", [w_rows, F], mybir.dt.float32, kind="ExternalInput")
    lidx = nc.dram_tensor("lidx", [GROUPS, P, 1], mybir.dt.int32, kind="ExternalInput")
    out = nc.dram_tensor("out", [P, GROUPS], mybir.dt.float32, kind="ExternalOutput")

    with tile.TileContext(nc) as tc:
        with (
            tc.tile_pool(name="big", bufs=2) as pool,
            tc.tile_pool(name="small", bufs=1) as spool,
        ):
            acc = spool.tile([P, GROUPS], mybir.dt.float32, tag="acc")
            for s in range(GROUPS):
                idx_t = spool.tile([P, 1], mybir.dt.int32, tag=f"idx{s}")
                nc.sync.dma_start(out=idx_t[:], in_=lidx[s])

                c_t = pool.tile([P, F], mybir.dt.float32, tag=f"c{s}")
                nc.gpsimd.indirect_dma_start(
                    out=c_t[:],
                    out_offset=None,
                    in_=cw[:],
                    in_offset=bass.IndirectOffsetOnAxis(ap=idx_t[:, :1], axis=0),
                )

                x_t = pool.tile([P, F], mybir.dt.float32, tag=f"x{s}")
                nc.sync.dma_start(out=x_t[:], in_=xg[s])

                d_t = pool.tile([P, F], mybir.dt.float32, tag=f"d{s}")
                nc.vector.tensor_tensor(
                    out=d_t[:], in0=x_t[:], in1=c_t[:], op=mybir.AluOpType.subtract
                )
                sq_t = pool.tile([P, F], mybir.dt.float32, tag=f"sq{s}")
                nc.scalar.activation(
                    out=sq_t[:],
                    in_=d_t[:],
                    func=mybir.ActivationFunctionType.Square,
                    accum_out=acc[:, s : s + 1],
                )

            clipped = spool.tile([P, GROUPS], mybir.dt.float32, tag="clipped")
            nc.vector.tensor_scalar(
                out=clipped[:],
                in0=acc[:],
                scalar1=1e-12,
                scalar2=1e12,
                op0=mybir.AluOpType.max,
                op1=mybir.AluOpType.min,
            )
            nc.sync.dma_start(out=out[:, :], in_=clipped[:])
    nc.compile()
    return nc


def kernel(x: np.ndarray, labels: np.ndarray, centers: np.ndarray) -> np.ndarray:
    global LAST_RESULTS
    from concourse.bass_utils import run_bass_kernel_spmd

    x = np.asarray(x, dtype=np.float32)
    centers = np.asarray(centers, dtype=np.float32)
    labels_np = np.asarray(labels).astype(np.int64)

    order = np.argsort(labels_np, kind="stable").reshape(N_CORES, SHARD)
    labs = labels_np[np.arange(B)][order]  # [N_CORES, SHARD], each row sorted
    lo = labs[:, 0]
    hi = labs[:, -1]
    w_rows = int((hi - lo).max()) + 1
    w_rows = max(w_rows, P)
    starts = np.minimum(lo, C - w_rows)

    key = w_rows
    if key not in _prog_cache:
        _prog_cache[key] = _build_program(w_rows)
    nc = _prog_cache[key]

    in_maps = []
    for k in range(N_CORES):
        xg = np.ascontiguousarray(x[order[k]].reshape(GROUPS, P, F))
        cw = np.ascontiguousarray(centers[starts[k] : starts[k] + w_rows])
        li = (labs[k] - starts[k]).astype(np.int32).reshape(GROUPS, P, 1)
        in_maps.append({"xg": xg, "cw": cw, "lidx": np.ascontiguousarray(li)})

    res = run_bass_kernel_spmd(nc, in_maps, core_ids=list(range(N_CORES)))
    LAST_RESULTS = res

    total = np.float32(0.0)
    for r in res.results:
        total += r["out"].sum(dtype=np.float32)
    loss = np.float32(total / np.float32(B))
    return np.asarray(loss, dtype=np.float32)


# revision 14
# speedup vs baseline: 1.0274x; 1.0274x over previous
# CenterLoss Trainium2 kernel.
#
# reference computes the full [B, C] squared-distance matrix but only reads
# the true-label entry of each row:
#   dist[i] = ||x[i] - centers[l_i]||^2
#   loss = mean(clip(dist, 1e-12, 1e12))
# so the kernel only needs a per-sample gather of center rows plus a fused
# (x-c)^2 row reduction - no matmul, memory-bound.
#
# Sharding (host side, inside kernel()):
#   - sort samples by label; each of the 8 cores gets 256 consecutive
#     samples of the sorted order, so its labels live in a contiguous
#     class range [start_k, start_k + W).
#   - core k inputs (fp16 to halve DMA bytes; rel error ~1e-5):
#     its 256 x-rows in gather layout, the centers window
#     rows [start_k : start_k+W) (same W on all cores - SPMD), and the
#     window-relative labels as int16 in dma_gather's wrapped layout.
#   - device: Q7 dma_gather of 2x128 center rows (16 DMA queues),
#     d = x - c then d*d sum via tensor_tensor_reduce on VectorE,
#     clip, write [128, GROUPS] f32 dists per core.
#   - host: sum the 8 partial vectors, divide by B.

import numpy as np

B = 2048
C = 16384
F = 2048
N_CORES = 8
SHARD = B // N_CORES  # 256 samples per core
P = 128
GROUPS = SHARD // P  # 2 gather groups of 128 rows

_prog_cache: dict = {}

# test.py introspection: the last BassKernelResults (exec_time_ns etc.)
LAST_RESULTS = None


def _build_program(w_rows: int):
    """One SPMD program, shared by all 8 cores; only the data differs."""
    from contextlib import ExitStack

    import concourse.bacc as bacc
    import concourse.bass as bass
    from concourse import mybir
    from concourse.library_config import mlp

    f16 = mybir.dt.float16
    f32 = mybir.dt.float32

    nc = bacc.Bacc("TRN2", debug=False)
    xg = nc.dram_tensor("xg", [GROUPS, P, F], f16, kind="ExternalInput")
    cw = nc.dram_tensor("cw", [w_rows, F], f16, kind="ExternalInput")
    lidx = nc.dram_tensor(
        "lidx", [P, GROUPS * (P // 16)], mybir.dt.int16, kind="ExternalInput"
    )
    out = nc.dram_tensor("out", [P, GROUPS], f32, kind="ExternalOutput")

    with (
        nc.Block() as block,
        nc.sbuf_tensor("idx_t", [P, GROUPS * (P // 16)], mybir.dt.int16) as idx_t,
        nc.sbuf_tensor("acc", [P, GROUPS], f32) as acc,
        nc.sbuf_tensor("clip_t", [P, GROUPS], f32) as clip_t,
        nc.sbuf_tensor("junk", [P, F], f16) as dummy,
        nc.semaphore("s_idx") as s_idx,
        nc.semaphore("s_out") as s_out,
        nc.semaphore("s_v") as s_v,
        nc.semaphore("s_d") as s_d,
        nc.semaphore("s_r") as s_r,
        ExitStack() as ctx,
    ):
        x_t = [
            ctx.enter_context(nc.sbuf_tensor(f"x{s}", [P, F], f16)) for s in range(GROUPS)
        ]
        c_t = [
            ctx.enter_context(nc.sbuf_tensor(f"c{s}", [P, 1, F], f16))
            for s in range(GROUPS)
        ]
        d_t = [
            ctx.enter_context(nc.sbuf_tensor(f"d{s}", [P, F], f16)) for s in range(GROUPS)
        ]
        s_x = [ctx.enter_context(nc.semaphore(f"s_x{s}")) for s in range(GROUPS)]
        s_g = [ctx.enter_context(nc.semaphore(f"s_g{s}")) for s in range(GROUPS)]

        @block.sync
        def _(sync: bass.BassEngine):
            sync.dma_start(out=idx_t[:], in_=lidx[:]).then_inc(s_idx, 16)
            for s in range(GROUPS):
                sync.dma_start(out=x_t[s][:], in_=xg[s]).then_inc(s_x[s], 16)
            sync.wait_ge(s_v, 1)
            sync.dma_start(out=out[:, :], in_=clip_t[:]).then_inc(s_out, 16)
            sync.wait_ge(s_out, 16)

        @block.gpsimd
        def _(gpsimd: bass.BassGpSimd):
            gpsimd.load_library(mlp)
            gpsimd.wait_ge(s_idx, 16)
            ncols = P // 16
            for s in range(GROUPS):
                gpsimd.dma_gather(
                    c_t[s][:],
                    cw[:],
                    idx_t[:, s * ncols : (s + 1) * ncols],
                    P,
                    P,
                    F,
                ).then_inc(s_g[s], 16)

        @block.vector
        def _(vector: bass.BassVectorEngine):
            for s in range(GROUPS):
                vector.wait_ge(s_g[s], 16)
                vector.wait_ge(s_x[s], 16)
                vector.tensor_tensor(
                    out=d_t[s][:],
                    in0=x_t[s][:],
                    in1=c_t[s][:, 0],
                    op=mybir.AluOpType.subtract,
                ).then_inc(s_d, 1)
            vector.wait_ge(s_r, GROUPS)
            vector.tensor_scalar(
                out=clip_t[:],
                in0=acc[:],
                scalar1=1e-12,
                scalar2=1e12,
                op0=mybir.AluOpType.max,
                op1=mybir.AluOpType.min,
            ).then_inc(s_v, 1)

        @block.scalar
        def _(scalar: bass.BassScalarEngine):
            for s in range(GROUPS):
                scalar.wait_ge(s_d, s + 1)
                scalar.activation(
                    out=dummy[:],
                    in_=d_t[s][:],
                    func=mybir.ActivationFunctionType.Square,
                    accum_out=acc[:, s : s + 1],
                ).then_inc(s_r, 1)

    nc.compile()
    return nc


def _wrap_idxs(lab_rel: np.ndarray) -> np.ndarray:
    """dma_gather wrapped index layout: index i of a 128-row group lives at
    [i % 16, i // 16], replicated across all 128 partitions; groups are
    stacked along the free dim."""
    cols = []
    for s in range(GROUPS):
        g = lab_rel[s * P : (s + 1) * P].astype(np.int16).reshape(P // 16, 16).T
        cols.append(np.tile(g, (P // 16, 1)))  # [128, P//16]
    return np.ascontiguousarray(np.concatenate(cols, axis=1))  # [128, GROUPS*P//16]


def kernel(x: np.ndarray, labels: np.ndarray, centers: np.ndarray) -> np.ndarray:
    global LAST_RESULTS
    from concourse.bass_utils import run_bass_kernel_spmd

    x = np.asarray(x)
    centers = np.asarray(centers)
    labels_np = np.asarray(labels).astype(np.int64)

    order = np.argsort(labels_np, kind="stable").reshape(N_CORES, SHARD)
    labs = labels_np[order]  # [N_CORES, SHARD], each row sorted
    lo = labs[:, 0]
    hi = labs[:, -1]
    w_rows = int((hi - lo).max()) + 1
    w_rows = max(w_rows, P)
    starts = np.minimum(lo, C - w_rows)

    x16 = x.astype(np.float16)
    c16 = centers.astype(np.float16)

    key = w_rows
    if key not in _prog_cache:
        _prog_cache[key] = _build_program(w_rows)
    nc = _prog_cache[key]

    in_maps = []
    for k in range(N_CORES):
        xg = np.ascontiguousarray(x16[order[k]].reshape(GROUPS, P, F))
        cw = np.ascontiguousarray(c16[starts[k] : starts[k] + w_rows])
        li = _wrap_idxs((labs[k] - starts[k]).astype(np.int16))
        in_maps.append({"xg": xg, "cw": cw, "lidx": li})

    res = run_bass_kernel_spmd(nc, in_maps, core_ids=list(range(N_CORES)))
    LAST_RESULTS = res

    total = np.float32(0.0)
    for r in res.results:
        total += r["out"].sum(dtype=np.float32)
    loss = np.float32(total / np.float32(B))
    return np.asarray(loss, dtype=np.float32)


# revision 17
# speedup vs baseline: 1.3099x; 1.2750x over previous
# CenterLoss Trainium2 kernel.
#
# reference computes the full [B, C] squared-distance matrix but only reads
# the true-label entry of each row:
#   dist[i] = ||x[i] - centers[l_i]||^2
#   loss = mean(clip(dist, 1e-12, 1e12))
# so the kernel only needs a per-sample gather of center rows plus a fused
# (x-c)^2 row reduction - no matmul, memory-bound.
#
# Sharding (host side, inside kernel()):
#   - sort samples by label; each of the 8 cores gets 256 consecutive
#     samples of the sorted order, so its labels live in a contiguous
#     class range [start_k, start_k + W).
#   - core k inputs (fp16 to halve DMA bytes; adds ~1e-6 rel error vs the
#     2e-2 tolerance): its 256 x-rows in gather layout, the centers window
#     rows [start_k : start_k+W) (same W on all cores - SPMD), and the
#     window-relative labels as int32.
#   - device (raw bacc, manual semaphores, 4 engines):
#       SP:  issue x loads, final out store
#       ACT: issue idx load first (frees the gather earliest), then
#            Square+accumulate per group
#       PL:  2 indirect-DMA gathers of 128 center rows each
#       DVE: d = x - c per group, final clip
#   - host: sum the 8 partial dist vectors, divide by B.

import numpy as np

B = 2048
C = 16384
F = 2048
N_CORES = 8
SHARD = B // N_CORES  # 256 samples per core
P = 128
GROUPS = SHARD // P  # 2 gather groups of 128 rows

_prog_cache: dict = {}

# test.py introspection: the last BassKernelResults (exec_time_ns etc.)
LAST_RESULTS = None


def _build_program(w_rows: int):
    """One SPMD program, shared by all 8 cores; only the data differs."""
    from contextlib import ExitStack

    import concourse.bacc as bacc
    import concourse.bass as bass
    from concourse import mybir

    f16 = mybir.dt.float16
    f32 = mybir.dt.float32

    nc = bacc.Bacc("TRN2", debug=False)
    xg = nc.dram_tensor("xg", [GROUPS, P, F], f16, kind="ExternalInput")
    cw = nc.dram_tensor("cw", [w_rows, F], f16, kind="ExternalInput")
    lidx = nc.dram_tensor("lidx", [P, GROUPS], mybir.dt.int32, kind="ExternalInput")
    out = nc.dram_tensor("out", [P, GROUPS], f32, kind="ExternalOutput")

    with (
        nc.Block(no_gpsimd_drain=True) as block,
        nc.sbuf_tensor("idx_t", [P, GROUPS], mybir.dt.int32) as idx_t,
        nc.sbuf_tensor("acc", [P, GROUPS], f32) as acc,
        nc.sbuf_tensor("clip_t", [P, GROUPS], f32) as clip_t,
        nc.sbuf_tensor("junk", [P, GROUPS, F], f16) as dummy,
        nc.semaphore("s_idx") as s_idx,
        nc.semaphore("s_out") as s_out,
        nc.semaphore("s_v") as s_v,
        nc.semaphore("s_d") as s_d,
        nc.semaphore("s_r") as s_r,
        ExitStack() as ctx,
    ):
        x_t = [
            ctx.enter_context(nc.sbuf_tensor(f"x{s}", [P, F], f16)) for s in range(GROUPS)
        ]
        c_t = [
            ctx.enter_context(nc.sbuf_tensor(f"c{s}", [P, F], f16)) for s in range(GROUPS)
        ]
        d_t = [
            ctx.enter_context(nc.sbuf_tensor(f"d{s}", [P, F], f16)) for s in range(GROUPS)
        ]
        s_x = [ctx.enter_context(nc.semaphore(f"s_x{s}")) for s in range(GROUPS)]
        s_g = [ctx.enter_context(nc.semaphore(f"s_g{s}")) for s in range(GROUPS)]

        @block.scalar
        def _(scalar: bass.BassScalarEngine):
            # idx load issued from ACT: it is the gather's gating input and
            # ACT is free earlier than SP in the preamble.
            scalar.dma_start(out=idx_t[:], in_=lidx[:]).then_inc(s_idx, 16)
            for s in range(GROUPS):
                scalar.wait_ge(s_d, s + 1)
                scalar.activation(
                    out=dummy[:, s],
                    in_=d_t[s][:],
                    func=mybir.ActivationFunctionType.Square,
                    accum_out=acc[:, s : s + 1],
                ).then_inc(s_r, 1)

        @block.sync
        def _(sync: bass.BassEngine):
            for s in range(GROUPS):
                sync.dma_start(out=x_t[s][:], in_=xg[s]).then_inc(s_x[s], 16)
            sync.wait_ge(s_v, 1)
            sync.dma_start(out=out[:, :], in_=clip_t[:]).then_inc(s_out, 16)
            sync.wait_ge(s_out, 16)

        @block.gpsimd
        def _(gpsimd: bass.BassGpSimd):
            gpsimd.wait_ge(s_idx, 16)
            for s in range(GROUPS):
                gpsimd.indirect_dma_start(
                    out=c_t[s][:],
                    out_offset=None,
                    in_=cw[:],
                    in_offset=bass.IndirectOffsetOnAxis(
                        ap=idx_t[:, s : s + 1], axis=0
                    ),
                ).then_inc(s_g[s], 16)

        @block.vector
        def _(vector: bass.BassVectorEngine):
            for s in range(GROUPS):
                vector.wait_ge(s_g[s], 16)
                vector.wait_ge(s_x[s], 16)
                vector.tensor_tensor(
                    out=d_t[s][:],
                    in0=x_t[s][:],
                    in1=c_t[s][:],
                    op=mybir.AluOpType.subtract,
                ).then_inc(s_d, 1)
            vector.wait_ge(s_r, GROUPS)
            vector.tensor_scalar(
                out=clip_t[:],
                in0=acc[:],
                scalar1=1e-12,
                scalar2=1e12,
                op0=mybir.AluOpType.max,
                op1=mybir.AluOpType.min,
            ).then_inc(s_v, 1)

    nc.compile()
    return nc


def kernel(x: np.ndarray, labels: np.ndarray, centers: np.ndarray) -> np.ndarray:
    global LAST_RESULTS
    from concourse.bass_utils import run_bass_kernel_spmd

    x = np.asarray(x)
    centers = np.asarray(centers)
    labels_np = np.asarray(labels).astype(np.int64)

    order = np.argsort(labels_np, kind="stable").reshape(N_CORES, SHARD)
    labs = labels_np[order]  # [N_CORES, SHARD], each row sorted
    lo = labs[:, 0]
    hi = labs[:, -1]
    w_rows = int((hi - lo).max()) + 1
    w_rows = max(w_rows, P)
    starts = np.minimum(lo, C - w_rows)

    x16 = x.astype(np.float16)
    c16 = centers.astype(np.float16)

    key = w_rows
    if key not in _prog_cache:
        _prog_cache[key] = _build_program(w_rows)
    nc = _prog_cache[key]

    in_maps = []
    for k in range(N_CORES):
        xg = np.ascontiguousarray(x16[order[k]].reshape(GROUPS, P, F))
        cw = np.ascontiguousarray(c16[starts[k] : starts[k] + w_rows])
        li = np.ascontiguousarray(
            (labs[k] - starts[k]).astype(np.int32).reshape(GROUPS, P).T
        )
        in_maps.append({"xg": xg, "cw": cw, "lidx": li})

    res = run_bass_kernel_spmd(nc, in_maps, core_ids=list(range(N_CORES)))
    LAST_RESULTS = res

    total = np.float32(0.0)
    for r in res.results:
        total += r["out"].sum(dtype=np.float32)
    loss = np.float32(total / np.float32(B))
    return np.asarray(loss, dtype=np.float32)
